# revision 1
# baseline (speedup 1.0000x reference)
"""Multi-head attention (B=2, S=4096, D=512, H=8) on 8 trn2 NeuronCores.

Sharding: head-parallel. Core i computes head i for BOTH batches (work per
head is proportional to that batch's valid_len, so pairing each head with
both batches balances the skewed valid_lens across cores). Each core
applies its row-slice of Wo on device and returns a full-shape partial;
the host sums the 8 partials (the tensor-parallel all-reduce, done in the
gather step).

Device dataflow per core (matmuls in fp16: same 1 cy/row PE rate as bf16
but 10 mantissa bits -> 5.9e-4 end-to-end error vs 4.8e-3 for bf16 and
3.1e-4 for float32r which runs at half rate; PSUM accumulates fp32):
  - projections:  Q^T[64,S] = Wq_h^T X^T,  K^T[64,vlp] likewise,
                  V[vlp,64] natural (plus a ones column -> softmax denom)
  - attention, per 512-wide q-block, streaming over 128-wide k-chunks:
        scoresT[k,q] = (K^T chunk as lhsT)^T @ Q^T block     (PE)
        E = exp(scoresT * 1/sqrt(hd) + mask_bias[k])         (ACT)
        outU[65,q]  += Vtilde_chunk^T @ E                    (PE, accum)
    row 64 of outU is the softmax denominator (ones column of Vtilde).
  - per q-block: outU -> SBUF, PE-transpose denominators to [q,1] layout,
    reciprocal, Wo matmul (lhsT = outU q-chunk), normalize with
    tensor_scalar_mul, DMA out.

Inputs beyond vl are never touched: K/V are projected only up to
vlp = ceil(vl/128)*128 and the boundary chunk is masked via the exp bias.
"""

import math
import os
from contextlib import ExitStack

import ml_dtypes
import numpy as np

import concourse.bass as bass
import concourse.mybir as mybir
import concourse.tile as tile
from concourse import bacc
from concourse import bass_utils

F32 = mybir.dt.float32
F32R = mybir.dt.float32r
MM_DT = F32R  # dtype of all matmul operands
EXP = mybir.ActivationFunctionType.Exp
NEG = -1.0e6

N_CORES = 8

# Problem shape (hardcoded per harness contract).
B_, S_, D_, H_ = 2, 4096, 512, 8
HD_ = D_ // H_


def _ceil_div(a, b):
    return (a + b - 1) // b


def _blocks(total, width):
    """[(offset, size), ...] covering `total` in chunks of `width`."""
    out = []
    off = 0
    while off < total:
        out.append((off, min(width, total - off)))
        off += width
    return out


def build_kernel(nc, cfg):
    """Emit the per-core kernel IR. cfg keys: S, D, HD, vlps (tuple per
    batch, each a multiple of 128)."""
    S, D, HD = cfg["S"], cfg["D"], cfg["HD"]
    mdt = {"f32r": F32R, "bf16": mybir.dt.bfloat16, "f16": mybir.dt.float16,
           "f32": F32}[cfg.get("dt", "f32r")]
    edt = mybir.dt.bfloat16 if cfg.get("ev_bf16") else mdt
    vlps = cfg["vlps"]
    B = len(vlps)
    ND = D // 128          # d-chunks
    scale = 1.0 / math.sqrt(HD)
    nch = [v // 128 for v in vlps]       # k-chunks per batch
    chbase = [sum(nch[:b]) for b in range(B)]   # chunk offset into mask/vbuf
    nch_tot = sum(nch)
    QB = 512                              # q-block width
    nqb = _ceil_div(S, QB)

    # ---- DRAM I/O ----
    qT = nc.dram_tensor("qT", [B, D, S], mdt, kind="ExternalInput").ap()
    kTs = [
        nc.dram_tensor(f"kT{b}", [D, vlps[b]], mdt, kind="ExternalInput").ap()
        for b in range(B)
    ]
    vTs = [
        nc.dram_tensor(f"vT{b}", [D, vlps[b]], mdt, kind="ExternalInput").ap()
        for b in range(B)
    ]
    wq = nc.dram_tensor("wq", [D, HD], mdt, kind="ExternalInput").ap()
    wk = nc.dram_tensor("wk", [D, HD], mdt, kind="ExternalInput").ap()
    wv = nc.dram_tensor("wv", [D, HD], mdt, kind="ExternalInput").ap()
    wo = nc.dram_tensor("wo", [HD, D], mdt, kind="ExternalInput").ap()
    mask = nc.dram_tensor("mask", [128, nch_tot], F32, kind="ExternalInput").ap()
    out = nc.dram_tensor("out", [B, S, D], F32, kind="ExternalOutput").ap()

    with tile.TileContext(nc) as tc, ExitStack() as ctx:
        consts = ctx.enter_context(tc.tile_pool(name="consts", bufs=1))
        xt = ctx.enter_context(tc.tile_pool(name="xt", bufs=2 * ND + 2))
        qkv = ctx.enter_context(tc.tile_pool(name="qkv", bufs=1))
        epool = ctx.enter_context(tc.tile_pool(name="e", bufs=4))
        ousb = ctx.enter_context(tc.tile_pool(name="ousb", bufs=3))
        stage = ctx.enter_context(tc.tile_pool(name="stage", bufs=3))
        small = ctx.enter_context(tc.tile_pool(name="small", bufs=2))
        ps_mm = ctx.enter_context(tc.tile_pool(name="ps_mm", bufs=2, space="PSUM"))
        ps_sc = ctx.enter_context(tc.tile_pool(name="ps_sc", bufs=4, space="PSUM"))
        ps_ou = ctx.enter_context(tc.tile_pool(name="ps_ou", bufs=2, space="PSUM"))

        # ---- constants ----
        wq_sb = consts.tile([128, ND, HD], mdt)
        wk_sb = consts.tile([128, ND, HD], mdt)
        wv_sb = consts.tile([128, ND, HD], mdt)
        for w_sb, w_ap in ((wq_sb, wq), (wk_sb, wk), (wv_sb, wv)):
            nc.sync.dma_start(out=w_sb, in_=w_ap.rearrange("(c p) h -> p c h", p=128))
        wo_sb = consts.tile([HD, D], mdt)
        nc.sync.dma_start(out=wo_sb, in_=wo)
        mask_sb = consts.tile([128, nch_tot], F32)
        nc.sync.dma_start(out=mask_sb, in_=mask)
        # unit2: [HD+1, 2] with row HD ones; extracts the denominator row of
        # outU as a [q, 2] column pair via one tiny matmul per q-chunk.
        unit2_f32 = consts.tile([HD + 1, 2], F32)
        nc.vector.memset(unit2_f32, 0.0)
        nc.vector.memset(unit2_f32[HD : HD + 1, :], 1.0)
        unit2 = consts.tile([HD + 1, 2], mdt)
        nc.vector.tensor_copy(unit2, unit2_f32)

        ones_stage = consts.tile([128, nch_tot, 1], F32)
        nc.vector.memset(ones_stage, 1.0)

        # ---- phase A: projections ----
        def load_xt_tiles(src_ap, soff, swidth):
            tiles = []
            for dc in range(ND):
                t = xt.tile([128, QB], mdt, tag="xt")
                nc.sync.dma_start(
                    out=t[:, :swidth],
                    in_=src_ap[dc * 128 : (dc + 1) * 128, soff : soff + swidth],
                )
                tiles.append(t)
            return tiles

        def emit():
          # persistent projected tensors: b0 rows 0:64, b1 rows 64:128
          qT_sb = qkv.tile([64 * B, S], mdt)
          kT_sb = qkv.tile([64 * B, max(vlps)], mdt)
          # V with appended ones column, per k-chunk: [128, chunk, HD+1]
          vbuf = qkv.tile([128, nch_tot, HD + 1], edt)
          nc.vector.tensor_copy(vbuf[:, :, HD : HD + 1], ones_stage)

          # ---- phase B: attention + Wo (called per batch so the
          # next batch's projection DMA streams underneath) ----
          def phase_b(b):
              r0 = b * 64
              recip = small.tile([128, 4 * nqb], F32)
              for qb in range(nqb):
                  qoff = qb * QB
                  qw = min(QB, S - qoff)
                  nq128 = qw // 128
                  ou = ps_ou.tile([HD + 1, QB], F32)
                  for kc in range(nch[b]):
                      ssc = ps_sc.tile([128, QB], F32)
                      nc.tensor.matmul(
                          ssc[:, :qw],
                          kT_sb[r0 : r0 + 64, kc * 128 : (kc + 1) * 128],
                          qT_sb[r0 : r0 + 64, qoff : qoff + qw],
                          start=True,
                          stop=True,
                      )
                      e = epool.tile([128, QB], edt)
                      if cfg.get("exp_on_dve"):
                          nc.vector.tensor_copy(e[:, :qw], ssc[:, :qw])
                      else:
                          nc.scalar.activation(
                              e[:, :qw],
                              ssc[:, :qw],
                              EXP,
                              bias=mask_sb[:, chbase[b] + kc : chbase[b] + kc + 1],
                              scale=scale,
                          )
                      nc.tensor.matmul(
                          ou[:, :qw],
                          vbuf[:, chbase[b] + kc, :],
                          e[:, :qw],
                          start=(kc == 0),
                          stop=(kc == nch[b] - 1),
                      )
                  ou_sb = ousb.tile([HD + 1, QB], mdt)
                  nc.vector.tensor_copy(ou_sb[:, :qw], ou[:, :qw])
                  st = stage.tile([128, QB // 128, D], F32)
                  for qi in range(nq128):
                      # denominator row -> [q, 2] column pair, then reciprocal
                      dps = ps_mm.tile([128, 2], F32, tag="mm")
                      nc.tensor.matmul(
                          dps,
                          ou_sb[0 : HD + 1, qi * 128 : (qi + 1) * 128],
                          unit2,
                          start=True,
                          stop=True,
                      )
                      col = qb * 4 + qi
                      nc.vector.reciprocal(recip[:, col : col + 1], dps[:, 0:1])
                      wps = ps_mm.tile([128, D], F32, tag="mm")
                      nc.tensor.matmul(
                          wps,
                          ou_sb[0:HD, qi * 128 : (qi + 1) * 128],
                          wo_sb,
                          start=True,
                          stop=True,
                      )
                      nc.vector.tensor_scalar_mul(
                          st[:, qi, :], wps, recip[:, col : col + 1]
                      )
                  out_eng = nc.gpsimd if cfg.get("swdge_out") else nc.sync
                  if cfg.get("no_out_dma"):
                      out_eng.dma_start(
                          out=out[b, qoff : qoff + 128, 0:1].rearrange(
                              "(q p) n -> p q n", p=128
                          ),
                          in_=st[:, :1, :1],
                      )
                  else:
                      out_eng.dma_start(
                          out=out[b, qoff : qoff + qw, :].rearrange(
                              "(q p) n -> p q n", p=128
                          ),
                          in_=st[:, :nq128, :],
                      )


          if True:
            for b in range(B):
              r0 = b * 64
              # Q^T  [64, S]
              for soff, sw in _blocks(S, QB):
                  tiles = load_xt_tiles(qT[b], soff, sw)
                  ps = ps_mm.tile([64, QB], F32, tag="mm")
                  for dc in range(ND):
                      nc.tensor.matmul(
                          ps[:, :sw],
                          wq_sb[:, dc, :],
                          tiles[dc][:, :sw],
                          start=(dc == 0),
                          stop=(dc == ND - 1),
                      )
                  nc.vector.tensor_copy(qT_sb[r0 : r0 + 64, soff : soff + sw], ps[:, :sw])
              # K^T  [64, vlp]
              for soff, sw in _blocks(vlps[b], QB):
                  tiles = load_xt_tiles(kTs[b], soff, sw)
                  ps = ps_mm.tile([64, QB], F32, tag="mm")
                  for dc in range(ND):
                      nc.tensor.matmul(
                          ps[:, :sw],
                          wk_sb[:, dc, :],
                          tiles[dc][:, :sw],
                          start=(dc == 0),
                          stop=(dc == ND - 1),
                      )
                  nc.vector.tensor_copy(kT_sb[r0 : r0 + 64, soff : soff + sw], ps[:, :sw])
              # V natural [vlp, HD] per 128-chunk
              for soff, sw in _blocks(vlps[b], QB):
                  tiles = load_xt_tiles(vTs[b], soff, sw)
                  for sub in range(sw // 128):
                      ps = ps_mm.tile([128, HD], F32, tag="mm")
                      for dc in range(ND):
                          nc.tensor.matmul(
                              ps,
                              tiles[dc][:, sub * 128 : (sub + 1) * 128],
                              wv_sb[:, dc, :],
                              start=(dc == 0),
                              stop=(dc == ND - 1),
                          )
                      kc = chbase[b] + (soff + sub * 128) // 128
                      nc.vector.tensor_copy(vbuf[:, kc, 0:HD], ps)
              if not cfg.get("split_phases", True):
                  phase_b(b)
          if cfg.get("split_phases", True):
              for b in range(B):
                  phase_b(b)

        for _ in range(cfg.get("repeat", 1)):
            emit()

    nc.compile()
    return nc


def prepare_in_maps(queries, keys, values, vls, Wq, Wk, Wv, Wo, vlps,
                    np_dt=np.float32):
    """Host-side layout prep: transposes, trims, per-core weight slices, mask."""
    HD = HD_
    queries, keys, values = (x.astype(np_dt) for x in (queries, keys, values))
    Wq, Wk, Wv, Wo = (x.astype(np_dt) for x in (Wq, Wk, Wv, Wo))
    qT = np.ascontiguousarray(queries.transpose(0, 2, 1))          # [B, D, S]
    kT = [np.ascontiguousarray(keys[b].T[:, : vlps[b]]) for b in range(B_)]
    vT = [np.ascontiguousarray(values[b].T[:, : vlps[b]]) for b in range(B_)]
    nch = [v // 128 for v in vlps]
    mask_np = np.zeros((128, sum(nch)), dtype=np.float32)
    cb = 0
    for b in range(B_):
        idx = np.arange(vlps[b]).reshape(nch[b], 128).T   # [128, nch]
        mask_np[:, cb : cb + nch[b]] = np.where(idx < vls[b], 0.0, NEG)
        cb += nch[b]

    in_maps = []
    for c in range(N_CORES):
        h0 = c * HD
        m = {
            "qT": qT,
            "wq": np.ascontiguousarray(Wq[:, h0 : h0 + HD]),
            "wk": np.ascontiguousarray(Wk[:, h0 : h0 + HD]),
            "wv": np.ascontiguousarray(Wv[:, h0 : h0 + HD]),
            "wo": np.ascontiguousarray(Wo[h0 : h0 + HD, :]),
            "mask": mask_np,
        }
        for b in range(B_):
            m[f"kT{b}"] = kT[b]
            m[f"vT{b}"] = vT[b]
        in_maps.append(m)
    return in_maps


_NC_CACHE = {}

# Matmul dtype for the graded kernel: "f32r" (accurate, ~2cy/row on HW) or
# "bf16" (fast). Overridable via env for experiments.
DEFAULT_DT = os.environ.get("KERNEL_DT", "f16")


def _get_nc(cfg_key):
    if cfg_key not in _NC_CACHE:
        S, D, HD, vlps, dt = cfg_key
        nc = bacc.Bacc(
            "TRN2",
            target_bir_lowering=False,
            debug=False,
            enable_asserts=False,
            num_devices=N_CORES,
        )
        build_kernel(nc, {"S": S, "D": D, "HD": HD, "vlps": vlps, "dt": dt})
        _NC_CACHE[cfg_key] = nc
    return _NC_CACHE[cfg_key]


LAST_RESULT = None  # BassKernelResults of the most recent kernel() call
LAST_IN_MAPS = None


def kernel(queries, keys, values, valid_lens, Wq, Wk, Wv, Wo, _trace=False):
    global LAST_RESULT, LAST_IN_MAPS
    queries = np.ascontiguousarray(np.asarray(queries, dtype=np.float32))
    keys = np.ascontiguousarray(np.asarray(keys, dtype=np.float32))
    values = np.ascontiguousarray(np.asarray(values, dtype=np.float32))
    Wq = np.ascontiguousarray(np.asarray(Wq, dtype=np.float32))
    Wk = np.ascontiguousarray(np.asarray(Wk, dtype=np.float32))
    Wv = np.ascontiguousarray(np.asarray(Wv, dtype=np.float32))
    Wo = np.ascontiguousarray(np.asarray(Wo, dtype=np.float32))
    vls = [int(v) for v in np.asarray(valid_lens).reshape(-1)]

    Bq, S, D = queries.shape
    assert (Bq, S, D) == (B_, S_, D_), (Bq, S, D)
    HD = HD_
    vlps = tuple(min(S, _ceil_div(max(v, 1), 128) * 128) for v in vls)

    dt = DEFAULT_DT
    nc = _get_nc((S, D, HD, vlps, dt))
    np_dt = {"bf16": ml_dtypes.bfloat16, "f16": np.float16}.get(dt, np.float32)
    in_maps = prepare_in_maps(
        queries, keys, values, vls, Wq, Wk, Wv, Wo, vlps, np_dt=np_dt
    )
    LAST_IN_MAPS = in_maps
    LAST_RESULT = bass_utils.run_bass_kernel_spmd(
        nc, in_maps, core_ids=list(range(N_CORES)), trace=_trace
    )
    acc = np.zeros((B_, S, D), dtype=np.float32)
    for r in LAST_RESULT.results:
        acc += r["out"]
    return acc



# revision 26
# speedup vs baseline: 1.2386x; 1.2386x over previous
"""Multi-head attention (B=2, S=4096, D=512, H=8) on 8 trn2 NeuronCores.

Sharding: head-parallel. Core i computes head i for BOTH batches. Each core
applies its row-slice of Wo on device and returns a full-shape f16 partial;
the host sums the 8 partials (the tensor-parallel all-reduce, done in the
gather step).

Device dataflow per core (matmuls in fp16; PSUM accumulates fp32):
  - projections: Wq/Wk head slices are column-DUPLICATED on host to [D, 128]
    so Q^T and K^T land in SBUF as [128, S]/[128, vlp] with identical top and
    bottom 64-row halves. V[vlp, 64] natural (plus a ones column so the AV
    matmul also produces the softmax denominator row).
  - attention, per 512-wide q-block, streaming over PAIRS of 128-wide
    k-chunks. The two score matmuls of a pair run CONCURRENTLY in the PE
    array via row tiling (contraction is only HD=64, so tile_position (0,0)
    and (64,0) each hold one 64x128 stationary chunk — ~2x score throughput):
        scoresT[k0,q] = kT[0:64,  chunk0]^T @ qT[0:64,  qblock]   -> psum half 0
        scoresT[k1,q] = kT[64:128,chunk1]^T @ qT[64:128,qblock]   -> psum half 1
    Interior chunks need no mask, so ONE activation computes
    exp(scale*scores) over the whole [128, 2, 512] pair tile (amortizes the
    ACT engine's per-instruction overhead; ACT is this kernel's critical
    engine). The boundary chunk (vl % 128 != 0) gets its own activation with
    the NEG bias column. Then per chunk:
        outU[65,q] += Vtilde_chunk^T @ E_chunk                    (PE, accum)
    row 64 of outU is the softmax denominator (ones column of Vtilde).
  - per q-block: outU -> SBUF f16, PE-transpose denominators to [q,1] layout,
    reciprocal, Wo matmul (lhsT = outU q-chunk), normalize with
    tensor_scalar_mul into f16, DMA out (f16 partial, halves out traffic).

Inputs beyond vl are never touched: K/V are projected only up to
vlp = ceil(vl/128)*128.
"""

import math
import os
from contextlib import ExitStack

import ml_dtypes
import numpy as np

import concourse.bass as bass
import concourse.mybir as mybir
import concourse.tile as tile
from concourse import bacc
from concourse import bass_utils

F32 = mybir.dt.float32
F16 = mybir.dt.float16
EXP = mybir.ActivationFunctionType.Exp
NEG = -1.0e6

N_CORES = 8

# Problem shape (hardcoded per harness contract).
B_, S_, D_, H_ = 2, 4096, 512, 8
HD_ = D_ // H_


def _ceil_div(a, b):
    return (a + b - 1) // b


def _blocks(total, width):
    out = []
    off = 0
    while off < total:
        out.append((off, min(width, total - off)))
        off += width
    return out


def build_kernel(nc, cfg):
    """Emit the per-core kernel IR. cfg keys: S, D, HD, vls (actual valid
    lens per batch), repeat."""
    S, D, HD = cfg["S"], cfg["D"], cfg["HD"]
    mdt = F16
    vls = cfg["vls"]
    B = len(vls)
    vlps = [min(S, _ceil_div(max(v, 1), 128) * 128) for v in vls]
    ND = D // 128
    scale = 1.0 / math.sqrt(HD)
    nch = [v // 128 for v in vlps]
    # chunks that need the NEG bias column (vl not 128-aligned)
    bnd = [vls[b] % 128 != 0 for b in range(B)]
    QB = 512
    assert S % QB == 0
    nqb = S // QB
    NQ = QB // 128  # 128-chunks per q-block

    # ---- DRAM I/O ----
    qT = nc.dram_tensor("qT", [B, D, S], mdt, kind="ExternalInput").ap()
    kTs = [
        nc.dram_tensor(f"kT{b}", [D, vlps[b]], mdt, kind="ExternalInput").ap()
        for b in range(B)
    ]
    vTs = [
        nc.dram_tensor(f"vT{b}", [D, vlps[b]], mdt, kind="ExternalInput").ap()
        for b in range(B)
    ]
    wqd = nc.dram_tensor("wqd", [D, 128], mdt, kind="ExternalInput").ap()
    wkd = nc.dram_tensor("wkd", [D, 128], mdt, kind="ExternalInput").ap()
    wv = nc.dram_tensor("wv", [D, HD], mdt, kind="ExternalInput").ap()
    wo = nc.dram_tensor("wo", [HD, D], mdt, kind="ExternalInput").ap()
    maskb = nc.dram_tensor("maskb", [128, B], F32, kind="ExternalInput").ap()
    out = nc.dram_tensor("out", [B, S, D], F16, kind="ExternalOutput").ap()

    with tile.TileContext(nc) as tc, ExitStack() as ctx:
        consts = ctx.enter_context(tc.tile_pool(name="consts", bufs=1))
        xt = ctx.enter_context(tc.tile_pool(name="xt", bufs=14))
        qkv = ctx.enter_context(tc.tile_pool(name="qkv", bufs=1))
        epool = ctx.enter_context(tc.tile_pool(name="e", bufs=4))
        ousb = ctx.enter_context(tc.tile_pool(name="ousb", bufs=3))
        stage = ctx.enter_context(tc.tile_pool(name="stage", bufs=3))
        small = ctx.enter_context(tc.tile_pool(name="small", bufs=2))
        ps_mm = ctx.enter_context(tc.tile_pool(name="ps_mm", bufs=2, space="PSUM"))
        ps_sc = ctx.enter_context(tc.tile_pool(name="ps_sc", bufs=2, space="PSUM"))
        ps_ou = ctx.enter_context(tc.tile_pool(name="ps_ou", bufs=2, space="PSUM"))

        # ---- constants (tiles only; DMAs are emitted by load_consts after
        # the first data loads so the first q/k tiles aren't queued behind
        # the weight transfers) ----
        wqd_sb = consts.tile([128, ND, 128], mdt)
        wkd_sb = consts.tile([128, ND, 128], mdt)
        wv_sb = consts.tile([128, ND, HD], mdt)
        wo_sb = consts.tile([HD, D], mdt)
        maskb_sb = consts.tile([128, B], F32)
        # unit2: [HD+1, 2] with row HD ones; extracts the denominator row of
        # outU as a [q, 2] column pair via one tiny matmul per q-chunk.
        unit2_f32 = consts.tile([HD + 1, 2], F32)
        nc.vector.memset(unit2_f32, 0.0)
        nc.vector.memset(unit2_f32[HD : HD + 1, :], 1.0)
        unit2 = consts.tile([HD + 1, 2], mdt)
        nc.vector.tensor_copy(unit2, unit2_f32)
        # warmup operand: the PE clock unthrottles only after ~3.4us of
        # sustained activity, so burn idle fill time on dummy matmuls.
        warm = consts.tile([64, QB], mdt)
        nc.vector.memset(warm, 0.0)

        max_nch = max(nch)
        ones_stage = consts.tile([128, max_nch, 1], F32)
        nc.vector.memset(ones_stage, 1.0)

        consts_loaded = [False]

        def load_consts():
            if consts_loaded[0]:
                return
            consts_loaded[0] = True
            for w_sb, w_ap in ((wqd_sb, wqd), (wkd_sb, wkd)):
                nc.sync.dma_start(
                    out=w_sb, in_=w_ap.rearrange("(c p) h -> p c h", p=128)
                )
            nc.sync.dma_start(out=wv_sb, in_=wv.rearrange("(c p) h -> p c h", p=128))
            nc.sync.dma_start(out=wo_sb, in_=wo)
            nc.sync.dma_start(out=maskb_sb, in_=maskb)

        def emit():
            # persistent projected tensors, per batch (dup row-halves)
            qT_sb = [
                qkv.tile([128, S], mdt, name=f"qT_sb{b}") for b in range(B)
            ]
            kT_sb = [
                qkv.tile([128, vlps[b]], mdt, name=f"kT_sb{b}") for b in range(B)
            ]
            vbuf = [
                qkv.tile([128, nch[b], HD + 1], mdt, name=f"vbuf{b}")
                for b in range(B)
            ]
            for b in range(B):
                nc.vector.tensor_copy(
                    vbuf[b][:, :, HD : HD + 1], ones_stage[:, : nch[b], :]
                )

            def load_xt(src_ap, soff, swidth):
                # One DMA per 512-block: [128, ND, sw] (partition = row within
                # d-chunk). Merging the per-chunk loads matters: each DMA
                # instruction costs ~625ns of serialized HWDGE descriptor-gen.
                t = xt.tile([128, ND, QB], mdt, tag="xt")
                nc.sync.dma_start(
                    out=t[:, :, :swidth],
                    in_=src_ap.rearrange("(c p) s -> p c s", p=128)[
                        :, :, soff : soff + swidth
                    ],
                )
                return t

            # ---- projection matmul pieces (DMA split out so loads can be
            # issued far ahead of the PE FIFO reaching the matmuls) ----
            def mm_k(b, soff, sw, t):
                ps = ps_mm.tile([128, QB], F32, tag="mm")
                for dc in range(ND):
                    nc.tensor.matmul(
                        ps[:, :sw],
                        wkd_sb[:, dc, :],
                        t[:, dc, :sw],
                        start=(dc == 0),
                        stop=(dc == ND - 1),
                    )
                nc.vector.tensor_copy(kT_sb[b][:, soff : soff + sw], ps[:, :sw])

            def mm_v(b, soff, sw, t):
                for sub in range(sw // 128):
                    ps = ps_mm.tile([128, HD], F32, tag="mm")
                    for dc in range(ND):
                        nc.tensor.matmul(
                            ps,
                            t[:, dc, sub * 128 : (sub + 1) * 128],
                            wv_sb[:, dc, :],
                            start=(dc == 0),
                            stop=(dc == ND - 1),
                        )
                    kc = (soff + sub * 128) // 128
                    nc.vector.tensor_copy(vbuf[b][:, kc, 0:HD], ps)

            def mm_q(b, soff, sw, t):
                ps = ps_mm.tile([128, QB], F32, tag="mm")
                for dc in range(ND):
                    nc.tensor.matmul(
                        ps[:, :sw],
                        wqd_sb[:, dc, :],
                        t[:, dc, :sw],
                        start=(dc == 0),
                        stop=(dc == ND - 1),
                    )
                nc.vector.tensor_copy(qT_sb[b][:, soff : soff + sw], ps[:, :sw])

            def proj_q_block(b, soff):
                t = load_xt(qT[b], soff, QB)
                mm_q(b, soff, QB, t)

            # ---- Wo + normalize + out DMA for one finished q-block.
            # mixed=True sends half the normalize-copies to the (then idle)
            # ACT engine to break the serial wps->normalize chain; used when
            # the tail is emitted into a short k-loop or at the very end.
            # split_dma=True overlaps the out DMA with the chain (final tail).
            def tail(b, qb, ou, mixed=False, split_dma=False):
                qoff = qb * QB
                ou_sb = ousb.tile([HD + 1, QB], mdt)
                nc.vector.tensor_copy(ou_sb, ou)
                recip = small.tile([128, NQ], F32, tag="recip")
                st = stage.tile([128, NQ, D], F16)
                for qi in range(NQ):
                    dps = ps_mm.tile([128, 2], F32, tag="mm")
                    nc.tensor.matmul(
                        dps,
                        ou_sb[0 : HD + 1, qi * 128 : (qi + 1) * 128],
                        unit2,
                        start=True,
                        stop=True,
                    )
                    nc.vector.reciprocal(recip[:, qi : qi + 1], dps[:, 0:1])
                    wps = ps_mm.tile([128, D], F32, tag="mm")
                    nc.tensor.matmul(
                        wps,
                        ou_sb[0:HD, qi * 128 : (qi + 1) * 128],
                        wo_sb,
                        start=True,
                        stop=True,
                    )
                    if mixed and qi % 2:
                        nc.scalar.activation(
                            st[:, qi, :],
                            wps,
                            mybir.ActivationFunctionType.Copy,
                            scale=recip[:, qi : qi + 1],
                        )
                    else:
                        nc.vector.tensor_scalar_mul(
                            st[:, qi, :], wps, recip[:, qi : qi + 1]
                        )
                    if split_dma:
                        nc.sync.dma_start(
                            out=out[
                                b, qoff + qi * 128 : qoff + (qi + 1) * 128, :
                            ].rearrange("(q p) n -> p q n", p=128),
                            in_=st[:, qi : qi + 1, :],
                        )
                if not split_dma:
                    nc.sync.dma_start(
                        out=out[b, qoff : qoff + QB, :].rearrange(
                            "(q p) n -> p q n", p=128
                        ),
                        in_=st,
                    )

            # ---- attention k-loop for one q-block. `inject` is a deferred
            # emission (previous q-block's Wo tail), emitted after the second
            # activation so the ACT engine has queued work covering the PE
            # time the tail instructions occupy. After every chunk group one
            # feeder (a projection piece for later data) is emitted too. ----
            def kloop(b, qb, inject):
                n = nch[b]
                n_int = n - 1 if bnd[b] else n  # interior (maskless) chunks
                qoff = qb * QB
                ou = ps_ou.tile([HD + 1, QB], F32)
                n_emitted = 0

                def av(kc, e_ap, first, last):
                    nc.tensor.matmul(
                        ou, vbuf[b][:, kc, :], e_ap, start=first, stop=last
                    )

                def maybe_inject():
                    nonlocal inject, n_emitted
                    n_emitted += 1
                    pop_feeder()
                    if inject is not None and n_emitted >= 2:
                        inject()
                        inject = None

                kc = 0
                while kc < n_int - 1:  # fused interior pairs
                    pp = ps_sc.tile([128, 2, QB], F32)
                    for i, half in enumerate((kc, kc + 1)):
                        r0 = i * 64
                        nc.tensor.matmul(
                            pp[:, i, :],
                            kT_sb[b][r0 : r0 + 64, half * 128 : half * 128 + 128],
                            qT_sb[b][r0 : r0 + 64, qoff : qoff + QB],
                            start=True,
                            stop=True,
                        )
                    e = epool.tile([128, 2, QB], mdt)
                    nc.scalar.activation(e, pp, EXP, scale=scale)
                    av(kc, e[:, 0, :], kc == 0, False)
                    av(kc + 1, e[:, 1, :], False, kc + 2 == n)
                    kc += 2
                    maybe_inject()
                while kc < n:  # leftover interior single + boundary chunk
                    pp = ps_sc.tile([128, 2, QB], F32)
                    nc.tensor.matmul(
                        pp[:, 0, :],
                        kT_sb[b][0:64, kc * 128 : kc * 128 + 128],
                        qT_sb[b][0:64, qoff : qoff + QB],
                        start=True,
                        stop=True,
                    )
                    e = epool.tile([128, 2, QB], mdt)
                    if kc >= n_int:
                        nc.scalar.activation(
                            e[:, 0, :],
                            pp[:, 0, :],
                            EXP,
                            bias=maskb_sb[:, b : b + 1],
                            scale=scale,
                        )
                    else:
                        nc.scalar.activation(e[:, 0, :], pp[:, 0, :], EXP, scale=scale)
                    av(kc, e[:, 0, :], kc == 0, kc + 1 == n)
                    kc += 1
                    maybe_inject()
                if inject is not None:  # tiny k-loop: emit anyway
                    inject()
                return ou

            # ---- schedule: the projection work list. Each item is a
            # (dma, mm) pair; DMAs are issued LOOKAHEAD items ahead of their
            # matmuls so the PE FIFO never head-of-line blocks on a load.
            # Ordered by when attention needs the data: q-block 0 and batch-0
            # k/v first, batch-1 entirely under batch-0's attention. ----
            items = []
            items.append((lambda: load_xt(qT[0], 0, QB), lambda t: mm_q(0, 0, QB, t)))
            for soff, sw in _blocks(vlps[0], QB):
                items.append(
                    (
                        lambda soff=soff, sw=sw: load_xt(kTs[0], soff, sw),
                        lambda t, soff=soff, sw=sw: mm_k(0, soff, sw, t),
                    )
                )
                items.append(
                    (
                        lambda soff=soff, sw=sw: load_xt(vTs[0], soff, sw),
                        lambda t, soff=soff, sw=sw: mm_v(0, soff, sw, t),
                    )
                )
            for qb in range(1, nqb):
                items.append(
                    (
                        lambda qb=qb: load_xt(qT[0], qb * QB, QB),
                        lambda t, qb=qb: mm_q(0, qb * QB, QB, t),
                    )
                )
            for soff, sw in _blocks(vlps[1], QB):
                items.append(
                    (
                        lambda soff=soff, sw=sw: load_xt(kTs[1], soff, sw),
                        lambda t, soff=soff, sw=sw: mm_k(1, soff, sw, t),
                    )
                )
                items.append(
                    (
                        lambda soff=soff, sw=sw: load_xt(vTs[1], soff, sw),
                        lambda t, soff=soff, sw=sw: mm_v(1, soff, sw, t),
                    )
                )
            for soff, sw in _blocks(S, QB):
                items.append(
                    (
                        lambda soff=soff: load_xt(qT[1], soff, QB),
                        lambda t, soff=soff: mm_q(1, soff, QB, t),
                    )
                )

            # Interleave batch-1's (short) q-block k-loops among batch-0's
            # rounds so every Wo tail hides under a following k-loop's ACT
            # work, instead of batch-1 running tail-serial at the end.
            def n_groups(b):
                ni = nch[b] - 1 if bnd[b] else nch[b]
                return ni // 2 + (ni % 2) + (1 if bnd[b] else 0)

            # ... and end on a batch-0 k-loop so the final tails hide under
            # its ACT work rather than running serial after everything.
            kl_seq = [(0, 0)]
            for qb in range(1, nqb - 1):
                kl_seq.append((0, qb))
                kl_seq.append((1, qb - 1))
            kl_seq += [(1, nqb - 2), (1, nqb - 1), (0, nqb - 1)]
            start_pop = {}
            acc = 0
            for b, qb in kl_seq:
                start_pop[(b, qb)] = acc
                acc += n_groups(b)

            # Pace the items across the k-loop groups so the PE never gets a
            # bunched run of projection matmuls (which starves ACT).
            # Deadline = the group-pop count after which the item may be
            # emitted.
            n_kv0 = 2 * len(_blocks(vlps[0], QB))
            n_kv1 = 2 * len(_blocks(vlps[1], QB))
            deadlines = [-1, -1, -1]  # q0b0 + first k/v block: primed
            deadlines += list(range(n_kv0 - 2))  # kv0 rest, one per group
            for j in range(1, nqb):  # q0 block j before k-loop (0, j)
                deadlines.append(max(n_kv0 - 2, start_pop[(0, j)] - 2))
            for r in range(n_kv1):  # kv1 before k-loop (1, 0)
                deadlines.append(max(n_kv0 - 2, start_pop[(1, 0)] - 2 * (n_kv1 - r)))
            for j in range(nqb):  # q1 block j before k-loop (1, j)
                deadlines.append(max(n_kv0 - 2, start_pop[(1, j)] - 4))
            order = sorted(range(len(items)), key=lambda i: (deadlines[i], i))
            sched = [(deadlines[i], items[i]) for i in order]

            LOOKAHEAD = 6
            state = {"nxt": 0, "dma": 0, "tiles": {}, "pops": 0}

            def emit_next():
                while state["dma"] < min(len(sched), state["nxt"] + 1 + LOOKAHEAD):
                    j = state["dma"]
                    state["tiles"][j] = sched[j][1][0]()
                    state["dma"] += 1
                i = state["nxt"]
                sched[i][1][1](state["tiles"].pop(i))
                state["nxt"] = i + 1

            def pop_feeder():
                p = state["pops"]
                state["pops"] += 1
                if state["nxt"] < len(sched) and sched[state["nxt"]][0] <= p:
                    emit_next()

            # prime: the first two data loads jump the DMA queue ahead of the
            # weight transfers; PE warms its clock on dummy matmuls while
            # they land; then q-block 0 and the first k/v block project.
            for j in range(2):
                state["tiles"][j] = sched[j][1][0]()
                state["dma"] = j + 1
            load_consts()
            wps_warm = ps_sc.tile([128, 2, QB], F32, tag="pp")
            for _ in range(7):
                nc.tensor.matmul(
                    wps_warm[:, 0, :], warm[:, 0:128], warm, start=True, stop=True
                )
            for _ in range(3):
                emit_next()

            pending = None  # deferred Wo-tail closure args
            for b, qb in kl_seq:
                inj = None
                if pending is not None:
                    pb, pqb, pou = pending
                    mixed = b == 1  # tail lands in a short k-loop: ACT idle
                    inj = lambda pb=pb, pqb=pqb, pou=pou, mx=mixed: tail(
                        pb, pqb, pou, mixed=mx
                    )
                ou = kloop(b, qb, inj)
                pending = (b, qb, ou)
            while state["nxt"] < len(sched):
                emit_next()
            tail(*pending, mixed=True, split_dma=True)

        for _ in range(cfg.get("repeat", 1)):
            emit()

    nc.compile()
    return nc


def prepare_in_maps(queries, keys, values, vls, Wq, Wk, Wv, Wo, vlps,
                    np_dt=np.float16):
    """Host-side layout prep: transposes, trims, per-core weight slices, mask."""
    HD = HD_
    queries, keys, values = (x.astype(np_dt) for x in (queries, keys, values))
    Wq, Wk, Wv, Wo = (x.astype(np_dt) for x in (Wq, Wk, Wv, Wo))
    qT = np.ascontiguousarray(queries.transpose(0, 2, 1))          # [B, D, S]
    kT = [np.ascontiguousarray(keys[b].T[:, : vlps[b]]) for b in range(B_)]
    vT = [np.ascontiguousarray(values[b].T[:, : vlps[b]]) for b in range(B_)]
    nch = [v // 128 for v in vlps]
    maskb_np = np.zeros((128, B_), dtype=np.float32)
    for b in range(B_):
        pos = (nch[b] - 1) * 128 + np.arange(128)
        maskb_np[:, b] = np.where(pos < vls[b], 0.0, NEG)

    in_maps = []
    for c in range(N_CORES):
        h0 = c * HD
        wq_h = Wq[:, h0 : h0 + HD]
        wk_h = Wk[:, h0 : h0 + HD]
        m = {
            "qT": qT,
            "wqd": np.ascontiguousarray(np.concatenate([wq_h, wq_h], axis=1)),
            "wkd": np.ascontiguousarray(np.concatenate([wk_h, wk_h], axis=1)),
            "wv": np.ascontiguousarray(Wv[:, h0 : h0 + HD]),
            "wo": np.ascontiguousarray(Wo[h0 : h0 + HD, :]),
            "maskb": maskb_np,
        }
        for b in range(B_):
            m[f"kT{b}"] = kT[b]
            m[f"vT{b}"] = vT[b]
        in_maps.append(m)
    return in_maps


_NC_CACHE = {}

DEFAULT_DT = "f16"


def _get_nc(cfg_key):
    if cfg_key not in _NC_CACHE:
        S, D, HD, vls = cfg_key
        nc = bacc.Bacc(
            "TRN2",
            target_bir_lowering=False,
            debug=False,
            enable_asserts=False,
            num_devices=N_CORES,
        )
        build_kernel(nc, {"S": S, "D": D, "HD": HD, "vls": vls})
        _NC_CACHE[cfg_key] = nc
    return _NC_CACHE[cfg_key]


LAST_RESULT = None  # BassKernelResults of the most recent kernel() call
LAST_IN_MAPS = None


def kernel(queries, keys, values, valid_lens, Wq, Wk, Wv, Wo, _trace=False):
    global LAST_RESULT, LAST_IN_MAPS
    queries = np.ascontiguousarray(np.asarray(queries, dtype=np.float32))
    keys = np.ascontiguousarray(np.asarray(keys, dtype=np.float32))
    values = np.ascontiguousarray(np.asarray(values, dtype=np.float32))
    Wq = np.ascontiguousarray(np.asarray(Wq, dtype=np.float32))
    Wk = np.ascontiguousarray(np.asarray(Wk, dtype=np.float32))
    Wv = np.ascontiguousarray(np.asarray(Wv, dtype=np.float32))
    Wo = np.ascontiguousarray(np.asarray(Wo, dtype=np.float32))
    vls = tuple(int(v) for v in np.asarray(valid_lens).reshape(-1))

    Bq, S, D = queries.shape
    assert (Bq, S, D) == (B_, S_, D_), (Bq, S, D)
    HD = HD_
    vlps = tuple(min(S, _ceil_div(max(v, 1), 128) * 128) for v in vls)

    nc = _get_nc((S, D, HD, vls))
    in_maps = prepare_in_maps(
        queries, keys, values, vls, Wq, Wk, Wv, Wo, vlps, np_dt=np.float16
    )
    LAST_IN_MAPS = in_maps
    LAST_RESULT = bass_utils.run_bass_kernel_spmd(
        nc, in_maps, core_ids=list(range(N_CORES)), trace=_trace
    )
    acc = np.zeros((B_, S, D), dtype=np.float32)
    for r in LAST_RESULT.results:
        acc += r["out"]
    return acc
